# revision 5
# baseline (speedup 1.0000x reference)
"""Trainium2 Bass kernel for nn_CRDLoss (CRD contrastive loss + memory bank update).

Strategy (8 NeuronCores, memory-bank rows sharded across cores):
  - Host pre-transposes/casts each bank shard to bf16 [D=128, rows] layout.
  - Each core computes all-pairs scores S'[b, row] = f^T-half @ bankT-chunk via
    PE matmuls (bf16), drains PSUM through ACT with a fused exp(s/T) (or DVE
    plain copy for a load-balance share of chunks, exp applied post-gather).
  - GpSimd indirect_copy gathers, per 16-partition group, the union of
    score columns actually referenced by (idx, contrast_idx); host-built
    binary weights select per-partition entries; DVE mul+reduce accumulates
    per-partition sums of exp(s/T)  (the Z sums), plus masked positive terms.
  - The 256 momentum-updated bank rows are computed on-device in f32.
  - Host combines the tiny per-core partials into the final loss analytically:
    sum_k log(c0/(x+c)) = B*K*log(c0/c) - sum(x)/c + O(sum (x/c)^2) (x << c;
    the quadratic term is ~1e-5 relative and provably negligible here).
"""

import numpy as np
import ml_dtypes

import concourse.bass as bass
import concourse.tile as tile
from concourse import bacc, mybir
from concourse import bass_utils

# ---------------- problem constants (hardcoded per contract) ----------------
B = 256
D = 128
K = 8192
KP1 = K + 1
N = 500000
NCORE = 8
NCE_T = 0.07
NCE_M = 0.5
EPS = 1e-7

CHUNK = 2048          # score columns per PSUM tile / extraction panel
MAXSLOTS = 1024       # ISA IndirectCopy dst element limit
DVE_DRAIN_MOD = 10**9  # DVE drains disabled: concurrent w/ gpsimd gather hard-faults
PROC_CHUNKS = None    # debug: limit number of processed chunks

BF16 = ml_dtypes.bfloat16


def _ceil16(x):
    return (int(x) + 15) // 16 * 16


# ---------------------------------------------------------------------------
# Host-side table construction
# ---------------------------------------------------------------------------
def _build_tables(idx_np, cidx_np, n_total, ncore, chunk):
    """Build per-core gather index tables, binary weights and positive masks.

    Returns dict with:
      idxtab  [ncore, 2, nchunk, 128, L//16] uint16
      wtab    [ncore, 2, nchunk, 128, L]     bf16  (binary entry weights)
      pmtab   [ncore, 2, nchunk, 128, 16]    bf16  (positive-slot diag masks)
      L       slots per gather (16 pos slots + padded union slots)
    """
    b_count = idx_np.shape[0]
    kp1 = cidx_np.shape[1] + 1
    r_per = n_total // ncore
    nchunk = (r_per + chunk - 1) // chunk

    full = np.concatenate([idx_np.astype(np.int64)[:, None],
                           cidx_np.astype(np.int64)], axis=1)      # [B, KP1]
    bb = np.repeat(np.arange(b_count, dtype=np.int64), kp1)
    nn = full.ravel()

    # unique (b, n) pairs with multiplicity
    pairkey = bb * n_total + nn
    uk, counts = np.unique(pairkey, return_counts=True)
    ub = uk // n_total
    un = uk % n_total

    core = un // r_per
    loc = un % r_per
    cq = loc // chunk
    win = loc % chunk
    half = ub // 128
    group = (ub % 128) // 16

    gkey = ((core * 2 + half) * nchunk + cq) * 8 + group           # group list id
    ngroups = ncore * 2 * nchunk * 8

    # per (gkey, win): rep = max multiplicity across the group's 16 lanes
    order = np.lexsort((win, gkey))
    gk_s = gkey[order]
    win_s = win[order]
    cnt_s = counts[order]
    nwkey_s = gk_s * chunk + win_s
    uniq_nw, first = np.unique(nwkey_s, return_index=True)
    rep = np.maximum.reduceat(cnt_s, first)                        # per nw
    g_of_nw = uniq_nw // chunk
    win_of_nw = uniq_nw % chunk

    # slot base per nw within its group list (nw sorted by (gkey, win))
    cumg = np.cumsum(rep) - rep                                    # global excl prefix
    g_first_vals, g_first_idx = np.unique(g_of_nw, return_index=True)
    # map each nw to its group's starting global prefix
    gstart = np.zeros(ngroups, dtype=np.int64)
    gstart[g_first_vals] = cumg[g_first_idx]
    base = cumg - gstart[g_of_nw]                                  # per nw

    # slots per group
    u_g = np.zeros(ngroups, dtype=np.int64)
    np.add.at(u_g, g_of_nw, rep)
    l_main = _ceil16(int(u_g.max()))
    L = 16 + l_main
    assert L <= MAXSLOTS, f"gather slot count {L} exceeds ISA limit"

    lc = L // 16
    idxtab = np.zeros((ncore, 2, nchunk, 128, lc), dtype=np.uint16)
    wtab = np.zeros((ncore, 2, nchunk, 128, L), dtype=BF16)
    pmtab = np.zeros((ncore, 2, nchunk, 128, 16), dtype=BF16)

    # ---- main slots: expand nw by rep ---------------------------------------
    tot = int(rep.sum())
    slot_win = np.repeat(win_of_nw, rep)
    slot_g = np.repeat(g_of_nw, rep)
    within = np.arange(tot, dtype=np.int64) - np.repeat(cumg, rep)
    slot_j = np.repeat(base, rep) + within                          # 0-based in group
    flat_i = 16 + slot_j
    sg_core = slot_g // (2 * nchunk * 8)
    sg_rem = slot_g % (2 * nchunk * 8)
    sg_half = sg_rem // (nchunk * 8)
    sg_rem2 = sg_rem % (nchunk * 8)
    sg_cq = sg_rem2 // 8
    sg_grp = sg_rem2 % 8
    row = 16 * sg_grp + (flat_i % 16)
    col = flat_i // 16
    idxtab[sg_core, sg_half, sg_cq, row, col] = slot_win.astype(np.uint16)

    # ---- binary weights: each (b, n) pair marks `count` consecutive slots ---
    nwkey_u = gkey * chunk + win
    pos_in_nw = np.searchsorted(uniq_nw, nwkey_u)
    base_u = base[pos_in_nw]
    tote = int(counts.sum())
    cume = np.cumsum(counts) - counts
    within_e = np.arange(tote, dtype=np.int64) - np.repeat(cume, counts)
    e_slot = np.repeat(base_u, counts) + within_e
    e_flat = 16 + e_slot
    e_core = np.repeat(core, counts)
    e_half = np.repeat(half, counts)
    e_cq = np.repeat(cq, counts)
    e_row = np.repeat(ub % 128, counts)
    wtab[e_core, e_half, e_cq, e_row, e_flat] = BF16(1.0)

    # ---- positives: slot i = lane holds win of lane's positive --------------
    pb = np.arange(b_count, dtype=np.int64)
    pn = idx_np.astype(np.int64)
    p_core = pn // r_per
    p_loc = pn % r_per
    p_cq = p_loc // chunk
    p_win = p_loc % chunk
    p_half = pb // 128
    p_row = pb % 128
    p_lane = pb % 16
    idxtab[p_core, p_half, p_cq, p_row, 0] = p_win.astype(np.uint16)
    pmtab[p_core, p_half, p_cq, p_row, p_lane] = BF16(1.0)

    return {"idxtab": idxtab, "wtab": wtab, "pmtab": pmtab, "L": L,
            "nchunk": nchunk, "r_per": r_per}


# ---------------------------------------------------------------------------
# Device kernel builder
# ---------------------------------------------------------------------------
def _build_kernel(L, nchunk, rpad):
    lc = L // 16
    f32 = mybir.dt.float32
    bf16 = mybir.dt.bfloat16
    u16 = mybir.dt.uint16
    AF = mybir.ActivationFunctionType
    OP = mybir.AluOpType
    AX = mybir.AxisListType

    nc = bacc.Bacc("TRN2", target_bir_lowering=False, debug=False,
                   num_devices=NCORE)

    b1t = nc.dram_tensor("b1t", [128, rpad], bf16, kind="ExternalInput").ap()
    b2t = nc.dram_tensor("b2t", [128, rpad], bf16, kind="ExternalInput").ap()
    ftT = nc.dram_tensor("ftT", [128, B], bf16, kind="ExternalInput").ap()
    fsT = nc.dram_tensor("fsT", [128, B], bf16, kind="ExternalInput").ap()
    idxt = nc.dram_tensor("idxt", [2, nchunk, 128, lc], u16, kind="ExternalInput").ap()
    wt = nc.dram_tensor("wt", [2, nchunk, 128, L], bf16, kind="ExternalInput").ap()
    pmt = nc.dram_tensor("pmt", [2, nchunk, 128, 16], bf16, kind="ExternalInput").ap()
    old1 = nc.dram_tensor("old1", [128, 2, 128], f32, kind="ExternalInput").ap()
    fsr = nc.dram_tensor("fsr", [128, 2, 128], f32, kind="ExternalInput").ap()
    old2 = nc.dram_tensor("old2", [128, 2, 128], f32, kind="ExternalInput").ap()
    ftr = nc.dram_tensor("ftr", [128, 2, 128], f32, kind="ExternalInput").ap()

    zsum_o = nc.dram_tensor("zsum", [128, 4], f32, kind="ExternalOutput").ap()
    pose_o = nc.dram_tensor("pose", [128, 4], f32, kind="ExternalOutput").ap()
    upd1_o = nc.dram_tensor("upd1", [128, 2, 128], f32, kind="ExternalOutput").ap()
    upd2_o = nc.dram_tensor("upd2", [128, 2, 128], f32, kind="ExternalOutput").ap()

    inv_t = 1.0 / NCE_T

    with tile.TileContext(nc) as tc:
        with (
            tc.tile_pool(name="fpool", bufs=1) as fpool,
            tc.tile_pool(name="rhs", bufs=4) as rhs_pool,
            tc.tile_pool(name="panel", bufs=4) as panel_pool,
            tc.tile_pool(name="psum", bufs=2, space="PSUM") as psum_pool,
            tc.tile_pool(name="tabs", bufs=3) as tab_pool,
            tc.tile_pool(name="ext", bufs=4) as ext_pool,
            tc.tile_pool(name="vt", bufs=4) as v_pool,
            tc.tile_pool(name="small", bufs=8) as small_pool,
            tc.tile_pool(name="accs", bufs=1) as acc_pool,
            tc.tile_pool(name="upd", bufs=1) as upd_pool,
        ):
            ft_tile = fpool.tile([128, B], bf16, tag="ftt")
            nc.sync.dma_start(ft_tile[:], ftT[:])
            fs_tile = fpool.tile([128, B], bf16, tag="fst")
            nc.sync.dma_start(fs_tile[:], fsT[:])

            acc_z = acc_pool.tile([128, 4], f32, tag="accz")
            nc.vector.memset(acc_z[:], 0.0)
            acc_p = acc_pool.tile([128, 4], f32, tag="accp")
            nc.vector.memset(acc_p[:], 0.0)

            nmm = CHUNK // 512

            nproc = nchunk if PROC_CHUNKS is None else min(PROC_CHUNKS, nchunk)
            for q in range(nproc):
                rhs = [None, None]
                for bank, src in ((0, b1t), (1, b2t)):
                    r = rhs_pool.tile([128, CHUNK], bf16, tag=f"rhs{bank}")
                    nc.sync.dma_start(r[:], src[:, q * CHUNK:(q + 1) * CHUNK])
                    rhs[bank] = r
                dve_drain = (q % DVE_DRAIN_MOD == 1)
                for half in (0, 1):
                    it = tab_pool.tile([128, lc], u16, tag="it")
                    nc.sync.dma_start(it[:], idxt[half, q])
                    wtl = tab_pool.tile([128, L], bf16, tag="wt")
                    nc.sync.dma_start(wtl[:], wt[half, q])
                    pm = tab_pool.tile([128, 16], bf16, tag="pm")
                    nc.sync.dma_start(pm[:], pmt[half, q])
                    for bank in (0, 1):
                        f_tile = ft_tile if bank == 0 else fs_tile
                        ps = psum_pool.tile([128, CHUNK], f32, tag="ps")
                        for m in range(nmm):
                            nc.tensor.matmul(
                                ps[:, m * 512:(m + 1) * 512],
                                f_tile[:, half * 128:(half + 1) * 128],
                                rhs[bank][:, m * 512:(m + 1) * 512],
                            )
                        panel = panel_pool.tile([128, CHUNK], f32, tag="panel")
                        if dve_drain:
                            nc.vector.tensor_copy(panel[:], ps[:])
                        else:
                            nc.scalar.activation(panel[:], ps[:], AF.Exp,
                                                 scale=inv_t)
                        ext = ext_pool.tile([128, L], f32, tag="ext")
                        nc.gpsimd.indirect_copy(
                            ext[:], panel[:], it[:],
                            i_know_ap_gather_is_preferred=True)
                        if dve_drain:
                            e2 = ext_pool.tile([128, L], f32, tag="ext2")
                            nc.scalar.activation(e2[:], ext[:], AF.Exp,
                                                 scale=inv_t)
                            ext = e2
                        col = bank * 2 + half
                        v = v_pool.tile([128, L], f32, tag="v")
                        nc.vector.tensor_tensor(v[:], ext[:], wtl[:], op=OP.mult)
                        r1 = small_pool.tile([128, 1], f32, tag="r1")
                        nc.vector.reduce_sum(r1[:], v[:], axis=AX.X)
                        nc.vector.tensor_tensor(
                            acc_z[:, col:col + 1], acc_z[:, col:col + 1], r1[:],
                            op=OP.add)
                        pv = small_pool.tile([128, 16], f32, tag="pv")
                        nc.vector.tensor_tensor(pv[:], ext[:, 0:16], pm[:],
                                                op=OP.mult)
                        r2 = small_pool.tile([128, 1], f32, tag="r2")
                        nc.vector.reduce_sum(r2[:], pv[:], axis=AX.X)
                        nc.vector.tensor_tensor(
                            acc_p[:, col:col + 1], acc_p[:, col:col + 1], r2[:],
                            op=OP.add)

            # ---- momentum update of the 256 touched rows (both banks) ------
            for tag, old_in, f_in, out_ap in (
                ("u1", old1, fsr, upd1_o), ("u2", old2, ftr, upd2_o)):
                to = upd_pool.tile([128, 2, 128], f32, tag=f"{tag}o")
                nc.sync.dma_start(to[:], old_in[:])
                tf = upd_pool.tile([128, 2, 128], f32, tag=f"{tag}f")
                nc.sync.dma_start(tf[:], f_in[:])
                ta = upd_pool.tile([128, 2, 128], f32, tag=f"{tag}a")
                nc.vector.tensor_tensor(ta[:], to[:], tf[:], op=OP.add)
                tb = upd_pool.tile([128, 2, 128], f32, tag=f"{tag}b")
                nc.vector.tensor_scalar_mul(tb[:], ta[:], 0.5)
                sq = upd_pool.tile([128, 2, 128], f32, tag=f"{tag}s")
                nc.vector.tensor_tensor(sq[:], tb[:], tb[:], op=OP.mult)
                ss = upd_pool.tile([128, 2], f32, tag=f"{tag}ss")
                nc.vector.reduce_sum(ss[:], sq[:], axis=AX.X)
                sn = upd_pool.tile([128, 2], f32, tag=f"{tag}sn")
                nc.scalar.sqrt(sn[:], ss[:])
                si = upd_pool.tile([128, 2], f32, tag=f"{tag}si")
                nc.vector.reciprocal(si[:], sn[:])
                tu = upd_pool.tile([128, 2, 128], f32, tag=f"{tag}u")
                for j in (0, 1):
                    nc.vector.tensor_scalar_mul(
                        tu[:, j, :], tb[:, j, :], si[:, j:j + 1])
                nc.sync.dma_start(out_ap[:], tu[:])

            nc.sync.dma_start(zsum_o[:], acc_z[:])
            nc.sync.dma_start(pose_o[:], acc_p[:])

    nc.compile()
    return nc


# ---------------------------------------------------------------------------
# Host orchestration
# ---------------------------------------------------------------------------
def _prep_inputs(f_s, f_t, memory_v1, memory_v2, idx, contrast_idx, tabs):
    r_per = tabs["r_per"]
    nchunk = tabs["nchunk"]
    rpad = nchunk * CHUNK
    idx64 = np.asarray(idx).astype(np.int64)

    ftT = np.ascontiguousarray(np.asarray(f_t).T).astype(BF16)
    fsT = np.ascontiguousarray(np.asarray(f_s).T).astype(BF16)

    def bank_shards(mem):
        mt = np.asarray(mem).T  # [D, N] view
        shards = []
        for c in range(NCORE):
            s = np.zeros((128, rpad), dtype=BF16)
            s[:, :r_per] = mt[:, c * r_per:(c + 1) * r_per].astype(BF16)
            shards.append(s)
        return shards

    b1 = bank_shards(memory_v1)
    b2 = bank_shards(memory_v2)

    def rows_lay(mem_rows):
        out = np.zeros((128, 2, 128), dtype=np.float32)
        for j in (0, 1):
            out[:, j, :] = mem_rows[j * 128:(j + 1) * 128]
        return out

    old1 = rows_lay(np.asarray(memory_v1)[idx64])
    old2 = rows_lay(np.asarray(memory_v2)[idx64])
    fsr = rows_lay(np.asarray(f_s))
    ftr = rows_lay(np.asarray(f_t))

    in_maps = []
    for c in range(NCORE):
        in_maps.append({
            "b1t": b1[c], "b2t": b2[c], "ftT": ftT, "fsT": fsT,
            "idxt": tabs["idxtab"][c], "wt": tabs["wtab"][c],
            "pmt": tabs["pmtab"][c],
            "old1": old1, "fsr": fsr, "old2": old2, "ftr": ftr,
        })
    return in_maps


def _combine(results, idx, memory_v1, memory_v2):
    idx64 = np.asarray(idx).astype(np.int64)
    zsum = np.stack([r["zsum"] for r in results]).astype(np.float64)  # [8,128,4]
    pose = np.stack([r["pose"] for r in results]).astype(np.float64)

    c_add = K * (1.0 / N) + EPS
    c0 = K * (1.0 / N)

    loss = 0.0
    for out_id, cols in (("v1", (2, 3)), ("v2", (0, 1))):
        # out_v1 <- bank v2 (f_s); out_v2 <- bank v1 (f_t)
        zs = zsum[:, :, cols[0]] + zsum[:, :, cols[1]]          # [8,128]
        z_tot = zs.sum()
        pe = np.concatenate([pose[:, :, cols[0]].sum(axis=0),
                             pose[:, :, cols[1]].sum(axis=0)])  # [256] by b
        z_const = z_tot * N / (B * KP1)
        x0 = pe / z_const
        p_term = np.sum(np.log(x0 / (x0 + c_add)))
        sum_neg_x = (z_tot - pe.sum()) / z_const
        n_term = (B * K) * np.log(c0 / c_add) - sum_neg_x / c_add
        loss += -(p_term + n_term) / B

    upd1 = results[0]["upd1"]
    upd2 = results[0]["upd2"]

    def unlay(u):
        rows = np.empty((256, 128), dtype=np.float32)
        for j in (0, 1):
            rows[j * 128:(j + 1) * 128] = u[:, j, :]
        return rows

    new_v1 = np.array(memory_v1, dtype=np.float32, copy=True)
    new_v2 = np.array(memory_v2, dtype=np.float32, copy=True)
    new_v1[idx64] = unlay(upd1)
    new_v2[idx64] = unlay(upd2)
    return np.float32(loss), new_v1, new_v2


_KERNEL_CACHE = {}


def kernel(f_s, f_t, memory_v1, memory_v2, idx, contrast_idx):
    idx_np = np.asarray(idx)
    cidx_np = np.asarray(contrast_idx)
    tabs = _build_tables(idx_np, cidx_np, N, NCORE, CHUNK)
    key = (tabs["L"], tabs["nchunk"])
    if key not in _KERNEL_CACHE:
        _KERNEL_CACHE[key] = _build_kernel(tabs["L"], tabs["nchunk"],
                                           tabs["nchunk"] * CHUNK)
    nc = _KERNEL_CACHE[key]
    in_maps = _prep_inputs(f_s, f_t, memory_v1, memory_v2, idx_np, cidx_np,
                           tabs)
    res = bass_utils.run_bass_kernel_spmd(nc, in_maps,
                                          core_ids=list(range(NCORE)))
    return _combine(res.results, idx_np, memory_v1, memory_v2)


# revision 10
# speedup vs baseline: 422.1638x; 422.1638x over previous
"""Trainium2 Bass kernel for nn_CRDLoss (CRD contrastive loss + memory bank update).

Strategy (8 NeuronCores, memory-bank rows row-sharded across cores):
  - Host pre-transposes/casts each bank shard to bf16 [D=128, rows] layout and
    builds a dense per-(batch,row) log-weight bias table:
        bias[b, n] = NCE_T * ln(multiplicity(b, n))   if (b, n) referenced
                   = -20                              otherwise
    so that  exp((s + bias)/T) = multiplicity * exp(s/T)  (0 when unreferenced,
    since exp(-20/0.07) underflows to exactly 0.0 in f32).
  - Each core computes all-pairs scores S'[b, row] = f^T-half @ bankT-chunk via
    PE matmuls (bf16) into PSUM; the VectorE drains PSUM fused with the dense
    bias add; ScalarE applies exp((s+bias)/NCE_T) with its per-partition
    accumulator producing the weighted row sums directly.
  - Per-core outputs are just the [128, 4] partial Z sums + the 256
    momentum-updated bank rows (computed on-device in f32).
  - Host combines partials analytically:
      sum_k log(c0/(x+c)) = B*K*log(c0/c) - sum(x)/c + O(sum (x/c)^2)
    (x << c always holds here; the quadratic term is ~2e-5 relative).
    The 256 positive scores are recomputed exactly on host (f64, 256x128 dots)
    for the positive log terms.
"""

import numpy as np
import ml_dtypes

import concourse.bass as bass
import concourse.tile as tile
from concourse import bacc, mybir
from concourse import bass_utils

# ---------------- problem constants (hardcoded per contract) ----------------
B = 256
D = 128
K = 8192
KP1 = K + 1
N = 500000
NCORE = 8
NCE_T = 0.07
NCE_M = 0.5
EPS = 1e-7

CHUNK = 2048          # score columns per PSUM tile
NEG_BIAS = -20.0      # log-weight for unreferenced entries -> exp == 0 in f32
PROC_CHUNKS = None    # debug: limit number of processed chunks
REPEAT = 1            # debug: repeat main loop (timing slope measurements)

BF16 = ml_dtypes.bfloat16


# ---------------------------------------------------------------------------
# Host-side dense log-weight bias table
# ---------------------------------------------------------------------------
def _build_tables(idx_np, cidx_np, n_total, ncore, chunk):
    b_count = idx_np.shape[0]
    r_per = n_total // ncore
    nchunk = (r_per + chunk - 1) // chunk
    rpad = nchunk * chunk

    full = np.concatenate([idx_np.astype(np.int64)[:, None],
                           cidx_np.astype(np.int64)], axis=1)      # [B, KP1]
    bb = np.repeat(np.arange(b_count, dtype=np.int64), full.shape[1])
    nn = full.ravel()
    pairkey = bb * n_total + nn
    uk, counts = np.unique(pairkey, return_counts=True)
    ub = uk // n_total
    un = uk % n_total

    core = un // r_per
    loc = un % r_per
    half = ub // 128
    row = ub % 128

    # dense bias [ncore, 2, 128, rpad] bf16
    wb = np.full((ncore, 2, 128, rpad), NEG_BIAS, dtype=BF16)
    vals = (NCE_T * np.log(counts.astype(np.float64))).astype(BF16)
    wb[core, half, row, loc] = vals
    return {"wb": wb, "nchunk": nchunk, "r_per": r_per, "rpad": rpad}


# ---------------------------------------------------------------------------
# Device kernel builder
# ---------------------------------------------------------------------------
def _build_kernel(nchunk, rpad):
    f32 = mybir.dt.float32
    bf16 = mybir.dt.bfloat16
    AF = mybir.ActivationFunctionType
    OP = mybir.AluOpType
    AX = mybir.AxisListType

    nc = bacc.Bacc("TRN2", target_bir_lowering=False, debug=False,
                   num_devices=NCORE)

    b1t = nc.dram_tensor("b1t", [128, rpad], bf16, kind="ExternalInput").ap()
    b2t = nc.dram_tensor("b2t", [128, rpad], bf16, kind="ExternalInput").ap()
    ftT = nc.dram_tensor("ftT", [128, B], bf16, kind="ExternalInput").ap()
    fsT = nc.dram_tensor("fsT", [128, B], bf16, kind="ExternalInput").ap()
    wbt = nc.dram_tensor("wbt", [2, 128, rpad], bf16, kind="ExternalInput").ap()
    old1 = nc.dram_tensor("old1", [128, 2, 128], f32, kind="ExternalInput").ap()
    fsr = nc.dram_tensor("fsr", [128, 2, 128], f32, kind="ExternalInput").ap()
    old2 = nc.dram_tensor("old2", [128, 2, 128], f32, kind="ExternalInput").ap()
    ftr = nc.dram_tensor("ftr", [128, 2, 128], f32, kind="ExternalInput").ap()

    zsum_o = nc.dram_tensor("zsum", [128, 4], f32, kind="ExternalOutput").ap()
    upd1_o = nc.dram_tensor("upd1", [128, 2, 128], f32, kind="ExternalOutput").ap()
    upd2_o = nc.dram_tensor("upd2", [128, 2, 128], f32, kind="ExternalOutput").ap()

    inv_t = 1.0 / NCE_T

    with tile.TileContext(nc) as tc:
        with (
            tc.tile_pool(name="fpool", bufs=1) as fpool,
            tc.tile_pool(name="rhs", bufs=4) as rhs_pool,
            tc.tile_pool(name="wb", bufs=4) as wb_pool,
            tc.tile_pool(name="sums", bufs=4) as sum_pool,
            tc.tile_pool(name="scr", bufs=4) as scr_pool,
            tc.tile_pool(name="psum", bufs=2, space="PSUM") as psum_pool,
            tc.tile_pool(name="small", bufs=8) as small_pool,
            tc.tile_pool(name="accs", bufs=1) as acc_pool,
            tc.tile_pool(name="upd", bufs=1) as upd_pool,
        ):
            ft_tile = fpool.tile([128, B], bf16, tag="ftt")
            nc.sync.dma_start(ft_tile[:], ftT[:])
            fs_tile = fpool.tile([128, B], bf16, tag="fst")
            nc.sync.dma_start(fs_tile[:], fsT[:])

            acc_z = acc_pool.tile([128, 4], f32, tag="accz")
            nc.vector.memset(acc_z[:], 0.0)

            nmm = CHUNK // 512
            nproc = nchunk if PROC_CHUNKS is None else min(PROC_CHUNKS, nchunk)
            for q_rep in range(REPEAT * nproc):
                q = q_rep % nproc
                rhs = [None, None]
                for bank, src in ((0, b1t), (1, b2t)):
                    r = rhs_pool.tile([128, CHUNK], bf16, tag=f"rhs{bank}")
                    nc.sync.dma_start(r[:], src[:, q * CHUNK:(q + 1) * CHUNK])
                    rhs[bank] = r
                for half in (0, 1):
                    wb = wb_pool.tile([128, CHUNK], bf16, tag="wb")
                    nc.sync.dma_start(
                        wb[:], wbt[half, :, q * CHUNK:(q + 1) * CHUNK])
                    for bank in (0, 1):
                        f_tile = ft_tile if bank == 0 else fs_tile
                        ps = psum_pool.tile([128, CHUNK], f32, tag="ps")
                        for m in range(nmm):
                            nc.tensor.matmul(
                                ps[:, m * 512:(m + 1) * 512],
                                f_tile[:, half * 128:(half + 1) * 128],
                                rhs[bank][:, m * 512:(m + 1) * 512],
                            )
                        st = sum_pool.tile([128, CHUNK], f32, tag="st")
                        nc.vector.tensor_tensor(st[:], ps[:], wb[:], op=OP.add)
                        scr = scr_pool.tile([128, CHUNK], f32, tag="scr")
                        r1 = small_pool.tile([128, 1], f32, tag="r1")
                        nc.scalar.activation(scr[:], st[:], AF.Exp,
                                             scale=inv_t, accum_out=r1[:])
                        col = bank * 2 + half
                        nc.vector.tensor_tensor(
                            acc_z[:, col:col + 1], acc_z[:, col:col + 1],
                            r1[:], op=OP.add)

            # ---- momentum update of the 256 touched rows (both banks) ------
            for tag, old_in, f_in, out_ap in (
                ("u1", old1, fsr, upd1_o), ("u2", old2, ftr, upd2_o)):
                to = upd_pool.tile([128, 2, 128], f32, tag=f"{tag}o")
                nc.sync.dma_start(to[:], old_in[:])
                tf = upd_pool.tile([128, 2, 128], f32, tag=f"{tag}f")
                nc.sync.dma_start(tf[:], f_in[:])
                ta = upd_pool.tile([128, 2, 128], f32, tag=f"{tag}a")
                nc.vector.tensor_tensor(ta[:], to[:], tf[:], op=OP.add)
                tb = upd_pool.tile([128, 2, 128], f32, tag=f"{tag}b")
                nc.vector.tensor_scalar_mul(tb[:], ta[:], 0.5)
                sq = upd_pool.tile([128, 2, 128], f32, tag=f"{tag}s")
                nc.vector.tensor_tensor(sq[:], tb[:], tb[:], op=OP.mult)
                ss = upd_pool.tile([128, 2], f32, tag=f"{tag}ss")
                nc.vector.reduce_sum(ss[:], sq[:], axis=AX.X)
                sn = upd_pool.tile([128, 2], f32, tag=f"{tag}sn")
                nc.scalar.sqrt(sn[:], ss[:])
                si = upd_pool.tile([128, 2], f32, tag=f"{tag}si")
                nc.vector.reciprocal(si[:], sn[:])
                tu = upd_pool.tile([128, 2, 128], f32, tag=f"{tag}u")
                for j in (0, 1):
                    nc.vector.tensor_scalar_mul(
                        tu[:, j, :], tb[:, j, :], si[:, j:j + 1])
                nc.sync.dma_start(out_ap[:], tu[:])

            nc.sync.dma_start(zsum_o[:], acc_z[:])

    nc.compile()
    return nc


# ---------------------------------------------------------------------------
# Host orchestration
# ---------------------------------------------------------------------------
def _prep_inputs(f_s, f_t, memory_v1, memory_v2, idx, contrast_idx, tabs):
    r_per = tabs["r_per"]
    rpad = tabs["rpad"]
    idx64 = np.asarray(idx).astype(np.int64)

    ftT = np.ascontiguousarray(np.asarray(f_t).T).astype(BF16)
    fsT = np.ascontiguousarray(np.asarray(f_s).T).astype(BF16)

    def bank_shards(mem):
        mt = np.asarray(mem).T  # [D, N] view
        shards = []
        for c in range(NCORE):
            s = np.zeros((128, rpad), dtype=BF16)
            s[:, :r_per] = mt[:, c * r_per:(c + 1) * r_per].astype(BF16)
            shards.append(s)
        return shards

    b1 = bank_shards(memory_v1)
    b2 = bank_shards(memory_v2)

    def rows_lay(mem_rows):
        out = np.zeros((128, 2, 128), dtype=np.float32)
        for j in (0, 1):
            out[:, j, :] = mem_rows[j * 128:(j + 1) * 128]
        return out

    old1 = rows_lay(np.asarray(memory_v1)[idx64])
    old2 = rows_lay(np.asarray(memory_v2)[idx64])
    fsr = rows_lay(np.asarray(f_s))
    ftr = rows_lay(np.asarray(f_t))

    in_maps = []
    for c in range(NCORE):
        in_maps.append({
            "b1t": b1[c], "b2t": b2[c], "ftT": ftT, "fsT": fsT,
            "wbt": tabs["wb"][c],
            "old1": old1, "fsr": fsr, "old2": old2, "ftr": ftr,
        })
    return in_maps


def _combine(results, f_s, f_t, memory_v1, memory_v2, idx):
    idx64 = np.asarray(idx).astype(np.int64)
    zsum = np.stack([r["zsum"] for r in results]).astype(np.float64)  # [8,128,4]

    c_add = K * (1.0 / N) + EPS
    c0 = K * (1.0 / N)

    # exact positive scores on host (f64)
    f_s64 = np.asarray(f_s, dtype=np.float64)
    f_t64 = np.asarray(f_t, dtype=np.float64)
    pos_v2 = np.exp(np.einsum(
        "bd,bd->b", np.asarray(memory_v1, np.float64)[idx64], f_t64) / NCE_T)
    pos_v1 = np.exp(np.einsum(
        "bd,bd->b", np.asarray(memory_v2, np.float64)[idx64], f_s64) / NCE_T)

    loss = 0.0
    for cols, pe in (((2, 3), pos_v1), (((0, 1)), pos_v2)):
        # out_v1 <- bank v2 (f_s) cols (2,3); out_v2 <- bank v1 (f_t) cols (0,1)
        z_tot = zsum[:, :, cols[0]].sum() + zsum[:, :, cols[1]].sum()
        z_const = z_tot * N / (B * KP1)
        x0 = pe / z_const
        p_term = np.sum(np.log(x0 / (x0 + c_add)))
        sum_neg_x = (z_tot - pe.sum()) / z_const
        n_term = (B * K) * np.log(c0 / c_add) - sum_neg_x / c_add
        loss += -(p_term + n_term) / B

    upd1 = results[0]["upd1"]
    upd2 = results[0]["upd2"]

    def unlay(u):
        rows = np.empty((256, 128), dtype=np.float32)
        for j in (0, 1):
            rows[j * 128:(j + 1) * 128] = u[:, j, :]
        return rows

    new_v1 = np.array(memory_v1, dtype=np.float32, copy=True)
    new_v2 = np.array(memory_v2, dtype=np.float32, copy=True)
    new_v1[idx64] = unlay(upd1)
    new_v2[idx64] = unlay(upd2)
    return np.float32(loss), new_v1, new_v2


_KERNEL_CACHE = {}


def kernel(f_s, f_t, memory_v1, memory_v2, idx, contrast_idx):
    idx_np = np.asarray(idx)
    cidx_np = np.asarray(contrast_idx)
    tabs = _build_tables(idx_np, cidx_np, N, NCORE, CHUNK)
    key = (tabs["nchunk"], tabs["rpad"])
    if key not in _KERNEL_CACHE:
        _KERNEL_CACHE[key] = _build_kernel(tabs["nchunk"], tabs["rpad"])
    nc = _KERNEL_CACHE[key]
    in_maps = _prep_inputs(f_s, f_t, memory_v1, memory_v2, idx_np, cidx_np,
                           tabs)
    res = bass_utils.run_bass_kernel_spmd(nc, in_maps,
                                          core_ids=list(range(NCORE)))
    return _combine(res.results, f_s, f_t, memory_v1, memory_v2, idx_np)


# revision 11
# speedup vs baseline: 746.8256x; 1.7690x over previous
"""Trainium2 Bass kernel for nn_CRDLoss (CRD contrastive loss + memory bank update).

Strategy (8 NeuronCores, memory-bank rows row-sharded across cores):
  - Host pre-transposes/casts each bank shard to bf16 [D=128, rows] layout and
    builds a dense per-(batch,row) log-weight bias table:
        bias[b, n] = NCE_T * ln(multiplicity(b, n))   if (b, n) referenced
                   = -20                              otherwise
    so that  exp((s + bias)/T) = multiplicity * exp(s/T)  (0 when unreferenced,
    since exp(-20/0.07) underflows to exactly 0.0 in f32).
  - Each core computes all-pairs scores S'[b, row] = f^T-half @ bankT-chunk via
    PE matmuls (bf16) into PSUM; the VectorE drains PSUM fused with the dense
    bias add; ScalarE applies exp((s+bias)/NCE_T) with its per-partition
    accumulator producing the weighted row sums directly.
  - Per-core outputs are just the [128, 4] partial Z sums + the 256
    momentum-updated bank rows (computed on-device in f32).
  - Host combines partials analytically:
      sum_k log(c0/(x+c)) = B*K*log(c0/c) - sum(x)/c + O(sum (x/c)^2)
    (x << c always holds here; the quadratic term is ~2e-5 relative).
    The 256 positive scores are recomputed exactly on host (f64, 256x128 dots)
    for the positive log terms.
"""

import numpy as np
import ml_dtypes

import concourse.bass as bass
import concourse.tile as tile
from concourse import bacc, mybir
from concourse import bass_utils

# ---------------- problem constants (hardcoded per contract) ----------------
B = 256
D = 128
K = 8192
KP1 = K + 1
N = 500000
NCORE = 8
NCE_T = 0.07
NCE_M = 0.5
EPS = 1e-7

CHUNK = 2048          # score columns per PSUM tile
NEG_BIAS = -20.0      # log-weight for unreferenced entries -> exp == 0 in f32
PROC_CHUNKS = None    # debug: limit number of processed chunks
REPEAT = 1            # debug: repeat main loop (timing slope measurements)

BF16 = ml_dtypes.bfloat16


# ---------------------------------------------------------------------------
# Host-side dense log-weight bias table
# ---------------------------------------------------------------------------
def _build_tables(idx_np, cidx_np, n_total, ncore, chunk):
    b_count = idx_np.shape[0]
    r_per = n_total // ncore
    nchunk = (r_per + chunk - 1) // chunk
    rpad = nchunk * chunk

    full = np.concatenate([idx_np.astype(np.int64)[:, None],
                           cidx_np.astype(np.int64)], axis=1)      # [B, KP1]
    bb = np.repeat(np.arange(b_count, dtype=np.int64), full.shape[1])
    nn = full.ravel()
    pairkey = bb * n_total + nn
    uk, counts = np.unique(pairkey, return_counts=True)
    ub = uk // n_total
    un = uk % n_total

    core = un // r_per
    loc = un % r_per
    half = ub // 128
    row = ub % 128

    # dense bias [ncore, 2, 128, rpad] bf16
    wb = np.full((ncore, 2, 128, rpad), NEG_BIAS, dtype=BF16)
    vals = (NCE_T * np.log(counts.astype(np.float64))).astype(BF16)
    wb[core, half, row, loc] = vals
    return {"wb": wb, "nchunk": nchunk, "r_per": r_per, "rpad": rpad}


# ---------------------------------------------------------------------------
# Device kernel builder
# ---------------------------------------------------------------------------
def _build_kernel(nchunk, rpad):
    f32 = mybir.dt.float32
    bf16 = mybir.dt.bfloat16
    AF = mybir.ActivationFunctionType
    OP = mybir.AluOpType
    AX = mybir.AxisListType

    nc = bacc.Bacc("TRN2", target_bir_lowering=False, debug=False,
                   num_devices=NCORE)

    b1t = nc.dram_tensor("b1t", [128, rpad], bf16, kind="ExternalInput").ap()
    ident = nc.dram_tensor("ident", [128, 128], bf16, kind="ExternalInput").ap()
    b2t = nc.dram_tensor("b2t", [128, rpad], bf16, kind="ExternalInput").ap()
    ftT = nc.dram_tensor("ftT", [128, B], bf16, kind="ExternalInput").ap()
    fsT = nc.dram_tensor("fsT", [128, B], bf16, kind="ExternalInput").ap()
    wbt = nc.dram_tensor("wbt", [2, 128, rpad], bf16, kind="ExternalInput").ap()
    old1 = nc.dram_tensor("old1", [128, 2, 128], f32, kind="ExternalInput").ap()
    fsr = nc.dram_tensor("fsr", [128, 2, 128], f32, kind="ExternalInput").ap()
    old2 = nc.dram_tensor("old2", [128, 2, 128], f32, kind="ExternalInput").ap()
    ftr = nc.dram_tensor("ftr", [128, 2, 128], f32, kind="ExternalInput").ap()

    zsum_o = nc.dram_tensor("zsum", [128, 4], f32, kind="ExternalOutput").ap()
    upd1_o = nc.dram_tensor("upd1", [128, 2, 128], f32, kind="ExternalOutput").ap()
    upd2_o = nc.dram_tensor("upd2", [128, 2, 128], f32, kind="ExternalOutput").ap()

    inv_t = 1.0 / NCE_T

    with tile.TileContext(nc) as tc:
        with (
            tc.tile_pool(name="fpool", bufs=1) as fpool,
            tc.tile_pool(name="rhs", bufs=4) as rhs_pool,
            tc.tile_pool(name="wb", bufs=4) as wb_pool,
            tc.tile_pool(name="scr", bufs=4) as scr_pool,
            tc.tile_pool(name="psum", bufs=2, space="PSUM") as psum_pool,
            tc.tile_pool(name="small", bufs=8) as small_pool,
            tc.tile_pool(name="accs", bufs=1) as acc_pool,
            tc.tile_pool(name="upd", bufs=1) as upd_pool,
        ):
            id_tile = fpool.tile([128, 128], bf16, tag="ident")
            nc.sync.dma_start(id_tile[:], ident[:])
            ft_tile = fpool.tile([128, B], bf16, tag="ftt")
            nc.sync.dma_start(ft_tile[:], ftT[:])
            fs_tile = fpool.tile([128, B], bf16, tag="fst")
            nc.sync.dma_start(fs_tile[:], fsT[:])

            acc_z = acc_pool.tile([128, 4], f32, tag="accz")
            nc.vector.memset(acc_z[:], 0.0)

            nmm = CHUNK // 512
            nproc = nchunk if PROC_CHUNKS is None else min(PROC_CHUNKS, nchunk)
            for q_rep in range(REPEAT * nproc):
                q = q_rep % nproc
                rhs = [None, None]
                for bank, src in ((0, b1t), (1, b2t)):
                    r = rhs_pool.tile([128, CHUNK], bf16, tag=f"rhs{bank}")
                    nc.sync.dma_start(r[:], src[:, q * CHUNK:(q + 1) * CHUNK])
                    rhs[bank] = r
                for half in (0, 1):
                    wb = wb_pool.tile([128, CHUNK], bf16, tag="wb")
                    nc.sync.dma_start(
                        wb[:], wbt[half, :, q * CHUNK:(q + 1) * CHUNK])
                    for bank in (0, 1):
                        f_tile = ft_tile if bank == 0 else fs_tile
                        ps = psum_pool.tile([128, CHUNK], f32, tag="ps")
                        for m in range(nmm):
                            nc.tensor.matmul(
                                ps[:, m * 512:(m + 1) * 512],
                                f_tile[:, half * 128:(half + 1) * 128],
                                rhs[bank][:, m * 512:(m + 1) * 512],
                                start=True, stop=False,
                            )
                        for m in range(nmm):
                            nc.tensor.matmul(
                                ps[:, m * 512:(m + 1) * 512],
                                id_tile[:],
                                wb[:, m * 512:(m + 1) * 512],
                                start=False, stop=True,
                            )
                        scr = scr_pool.tile([128, CHUNK], f32, tag="scr")
                        r1 = small_pool.tile([128, 1], f32, tag="r1")
                        nc.scalar.activation(scr[:], ps[:], AF.Exp,
                                             scale=inv_t, accum_out=r1[:])
                        col = bank * 2 + half
                        nc.vector.tensor_tensor(
                            acc_z[:, col:col + 1], acc_z[:, col:col + 1],
                            r1[:], op=OP.add)

            # ---- momentum update of the 256 touched rows (both banks) ------
            for tag, old_in, f_in, out_ap in (
                ("u1", old1, fsr, upd1_o), ("u2", old2, ftr, upd2_o)):
                to = upd_pool.tile([128, 2, 128], f32, tag=f"{tag}o")
                nc.sync.dma_start(to[:], old_in[:])
                tf = upd_pool.tile([128, 2, 128], f32, tag=f"{tag}f")
                nc.sync.dma_start(tf[:], f_in[:])
                ta = upd_pool.tile([128, 2, 128], f32, tag=f"{tag}a")
                nc.vector.tensor_tensor(ta[:], to[:], tf[:], op=OP.add)
                tb = upd_pool.tile([128, 2, 128], f32, tag=f"{tag}b")
                nc.vector.tensor_scalar_mul(tb[:], ta[:], 0.5)
                sq = upd_pool.tile([128, 2, 128], f32, tag=f"{tag}s")
                nc.vector.tensor_tensor(sq[:], tb[:], tb[:], op=OP.mult)
                ss = upd_pool.tile([128, 2], f32, tag=f"{tag}ss")
                nc.vector.reduce_sum(ss[:], sq[:], axis=AX.X)
                sn = upd_pool.tile([128, 2], f32, tag=f"{tag}sn")
                nc.scalar.sqrt(sn[:], ss[:])
                si = upd_pool.tile([128, 2], f32, tag=f"{tag}si")
                nc.vector.reciprocal(si[:], sn[:])
                tu = upd_pool.tile([128, 2, 128], f32, tag=f"{tag}u")
                for j in (0, 1):
                    nc.vector.tensor_scalar_mul(
                        tu[:, j, :], tb[:, j, :], si[:, j:j + 1])
                nc.sync.dma_start(out_ap[:], tu[:])

            nc.sync.dma_start(zsum_o[:], acc_z[:])

    nc.compile()
    return nc


# ---------------------------------------------------------------------------
# Host orchestration
# ---------------------------------------------------------------------------
def _prep_inputs(f_s, f_t, memory_v1, memory_v2, idx, contrast_idx, tabs):
    r_per = tabs["r_per"]
    rpad = tabs["rpad"]
    idx64 = np.asarray(idx).astype(np.int64)

    ftT = np.ascontiguousarray(np.asarray(f_t).T).astype(BF16)
    fsT = np.ascontiguousarray(np.asarray(f_s).T).astype(BF16)

    def bank_shards(mem):
        mt = np.asarray(mem).T  # [D, N] view
        shards = []
        for c in range(NCORE):
            s = np.zeros((128, rpad), dtype=BF16)
            s[:, :r_per] = mt[:, c * r_per:(c + 1) * r_per].astype(BF16)
            shards.append(s)
        return shards

    b1 = bank_shards(memory_v1)
    b2 = bank_shards(memory_v2)

    def rows_lay(mem_rows):
        out = np.zeros((128, 2, 128), dtype=np.float32)
        for j in (0, 1):
            out[:, j, :] = mem_rows[j * 128:(j + 1) * 128]
        return out

    old1 = rows_lay(np.asarray(memory_v1)[idx64])
    old2 = rows_lay(np.asarray(memory_v2)[idx64])
    fsr = rows_lay(np.asarray(f_s))
    ftr = rows_lay(np.asarray(f_t))

    in_maps = []
    for c in range(NCORE):
        in_maps.append({
            "b1t": b1[c], "b2t": b2[c], "ftT": ftT, "fsT": fsT,
            "ident": np.eye(128, dtype=BF16),
            "wbt": tabs["wb"][c],
            "old1": old1, "fsr": fsr, "old2": old2, "ftr": ftr,
        })
    return in_maps


def _combine(results, f_s, f_t, memory_v1, memory_v2, idx):
    idx64 = np.asarray(idx).astype(np.int64)
    zsum = np.stack([r["zsum"] for r in results]).astype(np.float64)  # [8,128,4]

    c_add = K * (1.0 / N) + EPS
    c0 = K * (1.0 / N)

    # exact positive scores on host (f64)
    f_s64 = np.asarray(f_s, dtype=np.float64)
    f_t64 = np.asarray(f_t, dtype=np.float64)
    pos_v2 = np.exp(np.einsum(
        "bd,bd->b", np.asarray(memory_v1, np.float64)[idx64], f_t64) / NCE_T)
    pos_v1 = np.exp(np.einsum(
        "bd,bd->b", np.asarray(memory_v2, np.float64)[idx64], f_s64) / NCE_T)

    loss = 0.0
    for cols, pe in (((2, 3), pos_v1), (((0, 1)), pos_v2)):
        # out_v1 <- bank v2 (f_s) cols (2,3); out_v2 <- bank v1 (f_t) cols (0,1)
        z_tot = zsum[:, :, cols[0]].sum() + zsum[:, :, cols[1]].sum()
        z_const = z_tot * N / (B * KP1)
        x0 = pe / z_const
        p_term = np.sum(np.log(x0 / (x0 + c_add)))
        sum_neg_x = (z_tot - pe.sum()) / z_const
        n_term = (B * K) * np.log(c0 / c_add) - sum_neg_x / c_add
        loss += -(p_term + n_term) / B

    upd1 = results[0]["upd1"]
    upd2 = results[0]["upd2"]

    def unlay(u):
        rows = np.empty((256, 128), dtype=np.float32)
        for j in (0, 1):
            rows[j * 128:(j + 1) * 128] = u[:, j, :]
        return rows

    new_v1 = np.array(memory_v1, dtype=np.float32, copy=True)
    new_v2 = np.array(memory_v2, dtype=np.float32, copy=True)
    new_v1[idx64] = unlay(upd1)
    new_v2[idx64] = unlay(upd2)
    return np.float32(loss), new_v1, new_v2


_KERNEL_CACHE = {}


def kernel(f_s, f_t, memory_v1, memory_v2, idx, contrast_idx):
    idx_np = np.asarray(idx)
    cidx_np = np.asarray(contrast_idx)
    tabs = _build_tables(idx_np, cidx_np, N, NCORE, CHUNK)
    key = (tabs["nchunk"], tabs["rpad"])
    if key not in _KERNEL_CACHE:
        _KERNEL_CACHE[key] = _build_kernel(tabs["nchunk"], tabs["rpad"])
    nc = _KERNEL_CACHE[key]
    in_maps = _prep_inputs(f_s, f_t, memory_v1, memory_v2, idx_np, cidx_np,
                           tabs)
    res = bass_utils.run_bass_kernel_spmd(nc, in_maps,
                                          core_ids=list(range(NCORE)))
    return _combine(res.results, f_s, f_t, memory_v1, memory_v2, idx_np)
